# revision 1
# baseline (speedup 1.0000x reference)
"""Trainium2 Bass kernel for nn_DS_Fusion_56495999811926 (dense_cnn).

Strategy: pure data parallelism — batch 16 sharded 2-per-core across 8
NeuronCores, weights replicated, no collectives.

Per-core program (C-layout: channels on partitions, pixels on free dim,
chunks of TN pixels):
  - All 1x1 convs as PE matmuls (float32r: 1 cyc/col at N=512).
  - BN folded into conv weights/biases on host; the residual-add's BN scale
    rides a diag() matmul accumulated into the same PSUM.
  - 96-row attention tensors use a gapped 112-row layout (blocks at 0:48 and
    64:112) so every engine op's partition base is 0/32/64/96; gap rows are
    zero-filled through zero-padded weight columns, never memset.
  - Per-pixel 4-way attention without partition reductions:
      logits    block-ones matmuls over (kk_exp + bias) * q_all
      max       duplicate pair-swapped logits via extra ones-matmuls, two
                aligned DVE maxes, then subtract via a -I8 matmul accumulated
                into the logits PSUM (k>=2 only; k0/k1 logit ranges are safe)
      softmax   ACT exp, denominators via block-ones matmul,
                reciprocal_approx_fast, broadcast-back matmul
      AV        broadcast matmul of att to the 112-row layout, multiply with
                v_all, block-sum folded into the emb1 conv weights
  - gelu (tanh form): Square/Tanh on ACT + 3 fused scalar_tensor_tensor ops
    (GPSIMD); the 0.5 is folded into emb2's weights.
  - Single ACT table set (exp_and_others) for the whole kernel.
"""
import numpy as np

EPS = 1e-5

B, C, H, W = 16, 48, 128, 128
N_CORES = 8
B_LOC = B // N_CORES
HW = H * W
TN = 1024

_prog_cache = {}

# gapped m-block row ranges in the 112-row layout
_BLK = [(0, 24), (24, 48), (64, 88), (88, 112)]


# ---------------------------------------------------------------- host math
def fold_params(inp):
    f32 = np.float32
    P = {}

    def bn_sc(pref):
        s = inp[pref + '_g'] / np.sqrt(inp[pref + '_v'] + EPS)
        t = inp[pref + '_b'] - inp[pref + '_m'] * s
        return s.astype(f32), t.astype(f32)

    def T(a):
        return np.ascontiguousarray(a.T.astype(f32))

    s_rb, t_rb = bn_sc('rb_bn')
    P['rb1T'] = T(s_rb[:, None] * inp['rb_w1'])             # [48,24]
    P['b_rb1'] = (s_rb * inp['rb_b1'] + t_rb)[:, None]      # [24,1]
    s_bn, t_bn = bn_sc('bn')
    P['rb2T'] = T(s_bn[:, None] * inp['rb_w2'])             # [24,48]
    P['b_rb2'] = (s_bn * inp['rb_b2'] + t_bn)[:, None]      # [48,1]
    P['sxv'] = s_bn[:, None].astype(f32)                    # [48,1]

    s_q, t_q = bn_sc('q_bn')
    qw = s_q[:, None] * inp['q_w']
    qb = s_q * inp['q_b'] + t_q
    s_v, t_v = bn_sc('v_bn')
    vw = s_v[:, None] * inp['v_w']
    vb = s_v * inp['v_b'] + t_v
    P['qTp'] = np.concatenate([T(qw), np.zeros((48, 16), f32)], 1)  # [48,64]
    P['vTp'] = np.concatenate([T(vw), np.zeros((48, 16), f32)], 1)  # [48,64]

    def gap_bias(b48):
        g = np.zeros((112, 1), f32)
        g[0:48, 0] = b48
        g[64:112, 0] = b48
        return g

    P['bq_g'] = gap_bias(qb)
    P['bv_g'] = gap_bias(vb)

    for i, pref in enumerate(('k1', 'k2')):
        s_k, t_k = bn_sc(pref + '_bn')
        kw = T(s_k[:, None] * inp[pref + '_w'])             # [48,24]
        kb = s_k * inp[pref + '_b'] + t_k                   # [24]
        kg = np.zeros((48, 112), f32)
        bg = np.zeros((112, 1), f32)
        for m in range(4):
            lo, hi = _BLK[m]
            kg[:, lo:hi] = kw
            bg[lo:hi, 0] = kb
        P[f'k{i + 1}expT'] = kg                             # [48,112]
        P[f'bk{i + 1}g'] = bg                               # [112,1]

    s_cf, t_cf = bn_sc('cf_bn')
    cw = s_cf[:, None] * inp['cf_w']                        # [48,96]
    P['cfaT'] = T(cw[:, :48])
    P['cfbT'] = T(cw[:, 48:])
    P['b_cf'] = (s_cf * inp['cf_b'] + t_cf)[:, None].astype(f32)

    w1 = inp['emb_w1'].astype(f32)                          # [24,48]
    e1a = np.zeros((112, 24), f32)
    e1b = np.zeros((112, 24), f32)
    for m in range(4):
        lo, hi = _BLK[m]
        e1a[lo:hi] = T(w1[:, :24])
        e1b[lo:hi] = T(w1[:, 24:])
    P['e1aT'] = e1a
    P['e1bT'] = e1b
    P['e1Tk0'] = T(w1)                                      # [48,24]
    P['b_e1'] = inp['emb_b1'][:, None].astype(f32)
    P['e2hT'] = T(inp['emb_w2'])                            # [24,48]
    P['b_e2'] = inp['emb_b2'][:, None].astype(f32)

    # attention constant matrices (gapped row space where 112-sized)
    for p in range(2):
        o = np.zeros((112, 8), f32)
        osw = np.zeros((112, 8), f32)
        for m in range(4):
            lo, hi = _BLK[m]
            o[lo:hi, 4 * p + m] = 1.0
            losw, hisw = _BLK[m ^ 1]
            osw[losw:hisw, 4 * p + m] = 1.0
        P[f'ones_p{p + 1}'] = o
        P[f'ones_sw_p{p + 1}'] = osw
        ae = np.zeros((8, 112), f32)
        for m in range(4):
            lo, hi = _BLK[m]
            ae[4 * p + m, lo:hi] = 1.0
        P[f'attexp{p + 1}T'] = ae
    sum4 = np.zeros((8, 2), f32)
    bc28 = np.zeros((2, 8), f32)
    for p in range(2):
        sum4[4 * p:4 * (p + 1), p] = 1.0
        bc28[p, 4 * p:4 * (p + 1)] = 1.0
    P['sum4T'] = sum4
    P['bc28T'] = bc28
    perm8 = np.zeros((8, 8), f32)
    for c, k in enumerate([2, 3, 0, 1, 6, 7, 4, 5]):
        perm8[k, c] = 1.0
    P['perm8T'] = perm8
    P['negI8'] = (-np.eye(8)).astype(f32)
    return P


# ---------------------------------------------------------------- program
def build_program(b_loc=B_LOC, hw=HW, tn=TN, use_f32r=False,
                  max_ks=(2, 3), repeat=1):
    import concourse.bacc as bacc
    import concourse.mybir as mybir
    from concourse import tile

    f32 = mybir.dt.float32
    f32r = mybir.dt.float32r
    mmf = f32r if use_f32r else f32
    A = mybir.ActivationFunctionType
    OP = mybir.AluOpType
    NH = tn // 512

    nc = bacc.Bacc(None, target_bir_lowering=False)

    wshapes = dict(rb1T=(48, 24), rb2T=(24, 48), sxv=(48, 1),
                   qTp=(48, 64), vTp=(48, 64), bq_g=(112, 1), bv_g=(112, 1),
                   k1expT=(48, 112), k2expT=(48, 112), bk1g=(112, 1),
                   bk2g=(112, 1), cfaT=(48, 48), cfbT=(48, 48),
                   e1aT=(112, 24), e1bT=(112, 24), e1Tk0=(48, 24),
                   e2hT=(24, 48), ones_p1=(112, 8), ones_p2=(112, 8),
                   ones_sw_p1=(112, 8), ones_sw_p2=(112, 8),
                   attexp1T=(8, 112), attexp2T=(8, 112), sum4T=(8, 2),
                   bc28T=(2, 8), perm8T=(8, 8), negI8=(8, 8),
                   b_rb1=(24, 1), b_rb2=(48, 1), b_e1=(24, 1), b_e2=(48, 1),
                   b_cf=(48, 1))
    BIAS_NAMES = {'sxv', 'bq_g', 'bv_g', 'bk1g', 'bk2g', 'b_rb1', 'b_rb2', 'b_e1',
                  'b_e2', 'b_cf'}

    def wdt(name):
        return f32 if name in BIAS_NAMES else mmf

    dram = {}
    for name, shp in wshapes.items():
        dram[name] = nc.declare_dram_parameter(name, list(shp), wdt(name),
                                               isOutput=False)
    x0_d = nc.declare_dram_parameter("x0", [b_loc, 48, hw], mmf, isOutput=False)
    x1_d = nc.declare_dram_parameter("x1", [b_loc, 48, hw], mmf, isOutput=False)
    out_d = nc.declare_dram_parameter("out", [b_loc, 48, hw], f32,
                                      isOutput=True)

    GA = float(np.float32(0.7978845608028654))
    GB = float(np.float32(0.7978845608028654 * 0.044715))

    nchunk = b_loc * hw // tn
    per_img = hw // tn

    with tile.TileContext(nc) as tc:
        with (tc.tile_pool(name="wp", bufs=1) as wp,
              tc.tile_pool(name="sp", bufs=2) as sp,
              tc.tile_pool(name="xp", bufs=6) as xp,
              tc.tile_pool(name="hp", bufs=4) as hp,
              tc.tile_pool(name="qp", bufs=2) as qp,
              tc.tile_pool(name="up", bufs=3) as up,
              tc.tile_pool(name="pp", bufs=4, space="PSUM") as pp):
            WT = {}
            for name, shp in wshapes.items():
                t = wp.tile(list(shp), wdt(name), name=f"w_{name}")
                nc.sync.dma_start(out=t[:, :], in_=dram[name][:, :])
                WT[name] = t

            def mm(ps, lhsT, rhs, start, stop):
                for hh in range(NH):
                    sl = slice(512 * hh, 512 * (hh + 1))
                    nc.tensor.matmul(ps[:, sl], lhsT, rhs[:, sl],
                                     start=start, stop=stop,
                                     skip_group_check=True)

            def psum(rows, name):
                return pp.tile([rows, tn], f32, tag="ps", name=name,
                               padded_shape=[128, tn])

            def gelu_emb2(ps_h, kk, ci):
                # exact erf-gelu on ACT; the ex-0.5 gelu factor folded into e2hT
                h_ = hp.tile([24, tn], mmf, tag="gh", name=f"gh_{ci}_{kk}")
                nc.scalar.activation(h_[:, :], ps_h[:, :], A.Gelu,
                                     bias=WT['b_e1'][:, 0:1])
                ps_la = psum(48, f"psla_{ci}_{kk}")
                mm(ps_la, WT['e2hT'][:, :], h_, True, True)
                la = hp.tile([48, tn], mmf, tag="la", name=f"la_{ci}_{kk}")
                nc.scalar.activation(la[:, :], ps_la[:, :], A.Identity,
                                     bias=WT['b_e2'][:, 0:1])
                return la

            from contextlib import nullcontext
            rep_ctx = tc.For_i(0, repeat, 1) if repeat > 1 else nullcontext()
            with rep_ctx:
              for ci in range(nchunk):
                bimg, off = ci // per_img, (ci % per_img) * tn
                xs = []
                for s, xd in enumerate((x0_d, x1_d)):
                    t = xp.tile([48, tn], mmf, tag="xs", name=f"x{s}_{ci}")
                    nc.sync.dma_start(out=t[:, :],
                                      in_=xd[bimg, :, off:off + tn])
                    xs.append(t)
                la = None
                for k in range(4):
                    # --- residual refinement (both streams) ---
                    for s in range(2):
                        ps_r = psum(24, f"psr{s}_{ci}_{k}")
                        mm(ps_r, WT['rb1T'][:, :], xs[s], True, True)
                        r_ = hp.tile([24, tn], mmf, tag="r",
                                     name=f"r{s}_{ci}_{k}")
                        nc.scalar.activation(r_[:, :], ps_r[:, :], A.Relu,
                                             bias=WT['b_rb1'][:, 0:1])
                        ps_x = psum(48, f"psx{s}_{ci}_{k}")
                        mm(ps_x, WT['rb2T'][:, :], r_, True, True)
                        sx = sp.tile([48, tn], f32, tag="sx",
                                     name=f"sx{s}_{ci}_{k}")
                        nc.vector.scalar_tensor_tensor(
                            sx[:, :], xs[s][:, :], WT['sxv'][:, 0:1],
                            ps_x[:, :], op0=OP.mult, op1=OP.add)
                        xn = xp.tile([48, tn], mmf, tag="xs",
                                     name=f"x{s}_{ci}_{k}")
                        nc.scalar.activation(xn[:, :], sx[:, :], A.Relu,
                                             bias=WT['b_rb2'][:, 0:1])
                        xs[s] = xn
                    # --- q/v convs; epilogues place streams into the
                    # gapped 112-row layout (out base 64 for stream 1) ---
                    ps_q0 = psum(64, f"psq0_{ci}_{k}")
                    ps_q1 = psum(48, f"psq1_{ci}_{k}")
                    ps_v0 = psum(64, f"psv0_{ci}_{k}")
                    ps_v1 = psum(48, f"psv1_{ci}_{k}")
                    mm(ps_q0, WT['qTp'][:, :], xs[0], True, True)
                    mm(ps_q1, WT['qTp'][:, 0:48], xs[1], True, True)
                    mm(ps_v0, WT['vTp'][:, :], xs[0], True, True)
                    mm(ps_v1, WT['vTp'][:, 0:48], xs[1], True, True)
                    qall = qp.tile([112, tn], mmf, tag="qall",
                                   name=f"q_{ci}_{k}")
                    vall = qp.tile([112, tn], f32, tag="vall",
                                   name=f"v_{ci}_{k}")
                    nc.scalar.activation(qall[0:64, :], ps_q0[:, :],
                                         A.Identity, bias=WT['bq_g'][0:64, 0:1])
                    nc.scalar.activation(qall[64:112, :], ps_q1[:, :],
                                         A.Identity,
                                         bias=WT['bq_g'][64:112, 0:1])
                    nc.vector.tensor_scalar(out=vall[0:64, :],
                                            in0=ps_v0[:, :],
                                            scalar1=WT['bv_g'][0:64, 0:1],
                                            scalar2=None, op0=OP.add)
                    nc.vector.tensor_scalar(out=vall[64:112, :],
                                            in0=ps_v1[:, :],
                                            scalar1=WT['bv_g'][64:112, 0:1],
                                            scalar2=None, op0=OP.add)
                    # --- k0: cross fusion + first emb ---
                    if k == 0:
                        ps_cf = psum(48, f"pscf_{ci}")
                        mm(ps_cf, WT['cfaT'][:, :], xs[0], True, False)
                        mm(ps_cf, WT['cfbT'][:, :], xs[1], False, True)
                        la0 = sp.tile([48, tn], mmf, tag="la0",
                                      name=f"la0_{ci}")
                        nc.scalar.activation(la0[:, :], ps_cf[:, :], A.Relu,
                                             bias=WT['b_cf'][:, 0:1])
                        ps_h0 = psum(24, f"psh0_{ci}")
                        mm(ps_h0, WT['e1Tk0'][:, :], la0, True, True)
                        la = gelu_emb2(ps_h0, "e", ci)
                    # --- attention ---
                    ps_log = psum(8, f"pslog_{ci}_{k}")
                    ts_ = []
                    for p in range(2):
                        ps_kk = psum(112, f"pskk{p}_{ci}_{k}")
                        mm(ps_kk, WT[f'k{p + 1}expT'][:, :], la, True, True)
                        t_ = hp.tile([112, tn], mmf, tag="t",
                                     name=f"t{p}_{ci}_{k}")
                        nc.vector.scalar_tensor_tensor(
                            t_[:, :], ps_kk[:, :], WT[f'bk{p + 1}g'][:, 0:1],
                            qall[:, :], op0=OP.add, op1=OP.mult)
                        ts_.append(t_)
                    do_max = k in max_ks
                    for p in range(2):
                        mm(ps_log, WT[f'ones_p{p + 1}'][:, :], ts_[p],
                           p == 0, p == 1 and not do_max)
                    if do_max:
                        ps_sw = psum(8, f"pssw_{ci}_{k}")
                        for p in range(2):
                            mm(ps_sw, WT[f'ones_sw_p{p + 1}'][:, :], ts_[p],
                               p == 0, p == 1)
                        sw_sb = sp.tile([8, tn], f32, tag="swsb",
                                        name=f"swsb_{ci}_{k}")
                        nc.scalar.activation(sw_sb[:, :], ps_sw[:, :],
                                             A.Identity)
                        mx1 = sp.tile([8, tn], mmf, tag="mx1",
                                      name=f"mx1_{ci}_{k}")
                        nc.vector.tensor_tensor(out=mx1[:, :],
                                                in0=ps_log[:, :],
                                                in1=sw_sb[:, :], op=OP.max)
                        ps_pm = psum(8, f"pspm_{ci}_{k}")
                        mm(ps_pm, WT['perm8T'][:, :], mx1, True, True)
                        mxf = sp.tile([8, tn], mmf, tag="mxf",
                                      name=f"mxf_{ci}_{k}")
                        nc.vector.tensor_tensor(out=mxf[:, :], in0=mx1[:, :],
                                                in1=ps_pm[:, :], op=OP.max)
                        mm(ps_log, WT['negI8'][:, :], mxf, False, True)
                    e_ = sp.tile([8, tn], mmf, tag="e", name=f"e_{ci}_{k}")
                    nc.scalar.activation(e_[:, :], ps_log[:, :], A.Exp)
                    ps_S = psum(2, f"psS_{ci}_{k}")
                    mm(ps_S, WT['sum4T'][:, :], e_, True, True)
                    rr = sp.tile([2, tn], mmf, tag="rr", name=f"rr_{ci}_{k}")
                    from concourse.dve_ops import (
                        RECIP_APPROX_FAST_CONSTS as _RC,
                        RECIPROCAL_APPROX_FAST as _RF)
                    nc.vector._custom_dve(_RF, out=rr[:, :], in0=ps_S[:, :],
                                          s0=_RC["s0"], s1=_RC["s1"],
                                          imm2=_RC["imm2"])
                    ps_rbc = psum(8, f"psrbc_{ci}_{k}")
                    mm(ps_rbc, WT['bc28T'][:, :], rr, True, True)
                    att = sp.tile([8, tn], mmf, tag="att", name=f"att_{ci}_{k}")
                    nc.vector.tensor_tensor(out=att[:, :], in0=e_[:, :],
                                            in1=ps_rbc[:, :], op=OP.mult)
                    us = []
                    for p in range(2):
                        ps_ae = psum(112, f"psae{p}_{ci}_{k}")
                        mm(ps_ae, WT[f'attexp{p + 1}T'][:, :], att, True, True)
                        u_ = up.tile([112, tn], mmf, tag="u",
                                     name=f"u{p}_{ci}_{k}")
                        nc.vector.tensor_tensor(out=u_[:, :], in0=ps_ae[:, :],
                                                in1=vall[:, :], op=OP.mult)
                        us.append(u_)
                    ps_h = psum(24, f"psh_{ci}_{k}")
                    mm(ps_h, WT['e1aT'][:, :], us[0], True, False)
                    mm(ps_h, WT['e1bT'][:, :], us[1], False, True)
                    la = gelu_emb2(ps_h, k, ci)
                nc.sync.dma_start(out=out_d[bimg, :, off:off + tn],
                                  in_=la[:, :].bitcast(f32))
    nc.compile()
    return nc


# ---------------------------------------------------------------- entry
def kernel(**inputs):
    from concourse.bass_utils import run_bass_kernel_spmd

    key = "full"
    if key not in _prog_cache:
        _prog_cache[key] = build_program()
    nc = _prog_cache[key]

    P = fold_params({k: np.asarray(v) for k, v in inputs.items()})
    x0 = np.asarray(inputs['x0'], np.float32).reshape(B, C, HW)
    x1 = np.asarray(inputs['x1'], np.float32).reshape(B, C, HW)
    in_maps = []
    for c in range(N_CORES):
        m = dict(P)
        m['x0'] = np.ascontiguousarray(x0[c * B_LOC:(c + 1) * B_LOC])
        m['x1'] = np.ascontiguousarray(x1[c * B_LOC:(c + 1) * B_LOC])
        in_maps.append(m)
    res = run_bass_kernel_spmd(nc, in_maps, list(range(N_CORES)))
    out = np.concatenate([res.results[c]['out'] for c in range(N_CORES)], 0)
    return out.reshape(B, C, H, W).astype(np.float32)


if __name__ == '__main__':
    import reference as R
    inputs = R.setup_inputs()
    expected = np.asarray(R.reference(**inputs))
    actual = kernel(**{k: np.asarray(v) for k, v in inputs.items()})
    denom = np.abs(expected).max()
    rel = np.abs(actual - expected).max() / denom
    print('rel err:', rel)

